# revision 5
# baseline (speedup 1.0000x reference)
# Trainium2 Bass kernel for nn_EquivariantCorrectionHead — v2.
#
# Structure per core (16384 samples): 32 macro-tiles of 512 samples (4 groups
# of 128). Per group, one stationary [89,128] (s|t feature-major) streams
# Wall [89, 2397] through PE into PSUM:
#   cols 0:1024    = |s|^2 id block (64) + 15 eigen-w's (shifted-PSD
#                    quadratic forms: h_w = sum_r (q'_r.s)^2 - c_w |s|^2)
#   cols 1024:2048 = 16 raw w's: z[(w,v)] = sum_u s_u W[u,(w,v)]
#   cols 2048:2397 = 1 raw w + a~ (stt/tst, 160) + E (Cbig proj, 125)
# Consume: ACT squares the eigen block out of PSUM; DVE multiplies the raw
# block by s (broadcast); grouped free-dim reduces on Pool/DVE build h1.
# Small bilinears (ht1/G/M/Q/o1/o2) stay sample-major with contiguous-inner
# APs, mults on Pool/DVE, packed 4-group reduces.
# tp2 transposes go through the DMA XBAR (dma_start_transpose) instead of PE.

import sys
from contextlib import ExitStack

import numpy as np

if "/opt/trn_rl_repo" not in sys.path:
    sys.path.insert(0, "/opt/trn_rl_repo")

import concourse.bass as bass
import concourse.mybir as mybir
import concourse.tile as tile
from concourse import bacc
from concourse.bass_utils import run_bass_kernel_spmd

B, NS, H = 131072, 64, 32
NCORES = 8
BPC = B // NCORES
P = 128
G = 4
MS = P * G                # 512 samples per macro-tile
NMACRO = BPC // MS        # 32

PW1_0 = float((NS * NS + 25.0) ** -0.5)
PW1_2 = float((5.0 / (10.0 * NS + 25.0)) ** 0.5)
PW2_2 = float((5.0 / (3.0 * H * H)) ** 0.5)
INV_S5 = float(5.0 ** -0.5)

F32 = mybir.dt.float32
F16 = mybir.dt.float16
AX = mybir.AxisListType
OP = mybir.AluOpType
AF = mybir.ActivationFunctionType

NEW = 23                  # eigen w's
NRW = 32 - NEW            # raw w's (w 15..31)
CE = 64 + NEW * 64        # 1024: id + eigen cols
CRA = 8 * 64              # raw cols in chunk A2
CRB = 64                  # raw w31 in ae chunk
CAE = 160 + 125           # a~ + E
NCOL = CE + CRA + CRB + CAE  # 2397


def _wigner3j_222():
    s2, s6 = np.sqrt(2.0), np.sqrt(6.0)
    M = np.zeros((5, 3, 3))
    M[0] = np.array([[0.0, 1, 0], [1, 0, 0], [0, 0, 0]]) / s2
    M[1] = np.array([[0.0, 0, 0], [0, 0, 1], [0, 1, 0]]) / s2
    M[2] = np.diag([-1.0, -1, 2]) / s6
    M[3] = np.array([[0.0, 0, 1], [0, 0, 0], [1, 0, 0]]) / s2
    M[4] = np.diag([1.0, -1, 0]) / s2
    C = np.einsum("aij,bjk,cki->abc", M, M, M)
    C = 0.5 * (C + C.transpose(1, 0, 2))
    return C / np.linalg.norm(C)


def prep_weights(w1_sss, w1_stt, w1_tst, w1_tts, w1_ttt, w2_stt, w2_tst, w2_ttt):
    C = _wigner3j_222()
    w1_sss = np.asarray(w1_sss, np.float64)
    w1_stt = np.asarray(w1_stt, np.float64)
    w1_tst = np.asarray(w1_tst, np.float64)
    w1_tts = np.asarray(w1_tts, np.float64)
    w1_ttt = np.asarray(w1_ttt, np.float64)
    w2_stt = np.asarray(w2_stt, np.float64)[:, :, 0]
    w2_tst = np.asarray(w2_tst, np.float64)[:, :, 0]
    w2_ttt = np.asarray(w2_ttt, np.float64)[:, :, 0]

    Wall = np.zeros((89, NCOL))
    cvec = np.zeros(32)
    # id block
    Wall[0:64, 0:64] = np.eye(64)
    # eigen w's 0..14
    for w in range(NEW):
        A = PW1_0 * w1_sss[:, :, w]
        S = 0.5 * (A + A.T)
        lam, Q = np.linalg.eigh(S)
        c = max(0.0, -lam.min()) + 1e-6
        cols = Q * np.sqrt(lam + c)
        Wall[0:64, 64 + w * 64: 64 + (w + 1) * 64] = cols
        cvec[w] = c
    # raw w's 15..31: col (w,v) = PW1_0 * W1sss[:, v, w]
    for i, w in enumerate(range(NEW, 32)):
        base = CE + i * 64
        Wall[0:64, base: base + 64] = PW1_0 * w1_sss[:, :, w]
    # a~ cols (w,r): rows u
    wb_ad = (PW1_2 * INV_S5) * (
        np.transpose(w1_stt, (0, 2, 1)) + np.transpose(w1_tst, (1, 2, 0))
    ).reshape(NS, H * 5)
    Wall[0:64, CE + CRA + CRB: CE + CRA + CRB + 160] = wb_ad
    # E cols: Cbig [(u',i)=25, (k,u,j)=125] on t-rows
    Cbig = np.zeros((25, 125))
    for u in range(5):
        for j in range(5):
            for k in range(5):
                Cbig[u * 5: u * 5 + 5, k * 25 + u * 5 + j] = C[:, j, k]
    Wall[64:89, CE + CRA + CRB + 160: NCOL] = Cbig

    # tp1 tts / tp2 maps
    Wtts = (PW1_0 * INV_S5) * w1_tts.reshape(25, H)           # [25, 32]
    wttt = PW1_2 * w1_ttt.reshape(25, H)
    WtttA = np.zeros((125, 128))
    WtttB = np.zeros((125, 32))
    for k in range(4):
        WtttA[k * 25: k * 25 + 25, k * 32: k * 32 + 32] = wttt
    WtttB[100:125, :] = wttt
    w2t = PW2_2 * w2_ttt
    W2A = np.zeros((128, 128))
    for i in range(4):
        W2A[i * 32: i * 32 + 32, i * 32: i * 32 + 32] = w2t
    W2B = w2t.copy()
    M2 = (PW2_2 * INV_S5) * (w2_stt + w2_tst.T)               # [32, 32]
    crepC = np.broadcast_to(
        np.transpose(C, (2, 0, 1)).reshape(1, 125), (P, 125)
    ).copy()
    crep_cw = np.broadcast_to(cvec.reshape(1, 32), (P, 32)).copy()

    f16 = lambda x: np.ascontiguousarray(x, np.float16)
    return {
        "Wall": f16(Wall),
        "Wtts": f16(Wtts),
        "WtttA": f16(WtttA),
        "WtttB": f16(WtttB),
        "W2A": f16(W2A),
        "W2B": f16(W2B),
        "M2": f16(M2),
        "CrepC": f16(crepC),
        "CrepW": f16(crep_cw),
    }


WEIGHT_SHAPES = {
    "Wall": (89, NCOL),
    "Wtts": (25, H),
    "WtttA": (125, 128),
    "WtttB": (125, 32),
    "W2A": (128, 128),
    "W2B": (32, 32),
    "M2": (32, 32),
    "CrepC": (P, 125),
    "CrepW": (P, 32),
}

# ws pack layout per group (512 f16): blocks of 128
#   [0:128)   M (125)
#   [128:256) G at 128:153 (25), h1 at 160:192 (32)
#   [256:384) ht1 feats 0:128
#   [384:416) ht1 feats 128:160  (block 3)
WS_M = 0
WS_G = 128
WS_H1 = 160
WS_HT1 = 256


def _tile_body(ctx: ExitStack, tc: tile.TileContext, io, n_macro: int):
    nc = tc.nc
    s_d, t_d, stT_d, out_d, wd = io["s"], io["t"], io["stT"], io["out"], io["w"]

    const = ctx.enter_context(tc.tile_pool(name="const", bufs=1))
    W = {}
    for name, shp in WEIGHT_SHAPES.items():
        W[name] = const.tile(list(shp), F16, tag=name, name=f"W_{name}")
        nc.sync.dma_start(W[name][:], wd[name])

    io_pool = ctx.enter_context(tc.tile_pool(name="io", bufs=2))
    zq = ctx.enter_context(tc.tile_pool(name="zq", bufs=2))
    sb = ctx.enter_context(tc.tile_pool(name="sb", bufs=2))
    fm = ctx.enter_context(tc.tile_pool(name="fm", bufs=2))
    zps = ctx.enter_context(tc.tile_pool(name="zps", bufs=1, space="PSUM"))
    tpsA = ctx.enter_context(tc.tile_pool(name="tpsA", bufs=2, space="PSUM"))
    tpsB = ctx.enter_context(tc.tile_pool(name="tpsB", bufs=1, space="PSUM"))

    ctx.enter_context(nc.allow_low_precision("fp16 intermediates fit the 2e-2 budget"))

    for im in range(n_macro):
        r0 = im * MS
        # ---- loads ----
        st4 = io_pool.tile([P, G, 96], F16, tag="st4")
        nc.sync.dma_start(st4[:, :, 0:64], s_d[r0: r0 + MS, :].rearrange("(g p) u -> p g u", g=G))
        nc.sync.dma_start(st4[:, :, 64:89], t_d[r0: r0 + MS, :].rearrange("(g p) u -> p g u", g=G))
        stT4 = io_pool.tile([89, MS], F16, tag="stT4")
        nc.sync.dma_start(stT4[:], stT_d[:, r0: r0 + MS])

        zsqE = zq.tile([P, G, CE], F16, tag="zsqE")      # squares (id+eigen)
        zsqR = zq.tile([P, G, CRA + CRB], F16, tag="zsqR")  # raw z*s
        ae4 = sb.tile([P, G, 285], F16, tag="ae4")

        for g in range(G):
            lhs = stT4[:, g * P: (g + 1) * P]
            zp = zps.tile([P, NCOL], F32, tag="zp")
            for c0 in range(0, 2048, 512):
                nc.tensor.matmul(zp[:, c0: c0 + 512], lhs, W["Wall"][:, c0: c0 + 512], start=True, stop=True)
            nc.tensor.matmul(zp[:, 2048:NCOL], lhs, W["Wall"][:, 2048:NCOL], start=True, stop=True)

            # consume: ACT squares the id+eigen chunk
            nc.scalar.activation(zsqE[:, g, 0:1024], zp[:, 0:1024], AF.Square)
            nc.scalar.activation(zsqE[:, g, 1024:1536], zp[:, 1024:1536], AF.Square)
            # DVE: raw chunk z * s (broadcast middle axis)
            s3a = st4[:, g, 0:64].unsqueeze(1).to_broadcast((P, 9, 64))
            nc.vector.tensor_tensor(
                zsqR[:, g, :].rearrange("p (w v) -> p w v", w=9),
                zp[:, 1536:2112].rearrange("p (w v) -> p w v", w=9), s3a, OP.mult)
            # ACT: evict a~ + E
            nc.scalar.activation(ae4[:, g, :], zp[:, 2112:NCOL], AF.Copy)

        # ---- h1: grouped reduces + shift fix ----
        tmp_e = sb.tile([P, G, 33], F32, tag="tmp_e")
        # Pool halves the 64-wide groups, DVE reduces the remaining 32
        zhE = zq.tile([P, G, 24, 32], F16, tag="zhE")
        zE4 = zsqE[:].rearrange("p g (w h v) -> p (g w) h v", h=2, v=32)
        nc.gpsimd.tensor_tensor(
            zhE[:].rearrange("p g w v -> p (g w) v"), zE4[:, :, 0, :], zE4[:, :, 1, :], OP.add)
        nc.vector.tensor_reduce(
            tmp_e[:, :, 0:24], zhE[:].rearrange("p g w v -> p (g w) v"),
            axis=AX.X, op=OP.add)
        zhR = zq.tile([P, G, 9, 32], F16, tag="zhR")
        zR4 = zsqR[:].rearrange("p g (w h v) -> p (g w) h v", h=2, v=32)
        nc.gpsimd.tensor_tensor(
            zhR[:].rearrange("p g w v -> p (g w) v"), zR4[:, :, 0, :], zR4[:, :, 1, :], OP.add)
        nc.vector.tensor_reduce(
            tmp_e[:, :, 24:33], zhR[:].rearrange("p g w v -> p (g w) v"),
            axis=AX.X, op=OP.add)

        ws = sb.tile([P, G, 512], F16, tag="ws")
        # h1 = tmp_e[1:33] - c_w * S2
        tmp2 = sb.tile([P, G, 32], F32, tag="tmp2")
        s2b = tmp_e[:, :, 0:1].to_broadcast((P, G, 32))
        cwb = W["CrepW"][:].unsqueeze(1).to_broadcast((P, G, 32))
        nc.vector.tensor_tensor(tmp2[:], s2b, cwb, OP.mult)
        nc.vector.tensor_tensor(ws[:, :, WS_H1: WS_H1 + 32], tmp_e[:, :, 1:33], tmp2[:], OP.subtract)

        # ---- small bilinears (sample-major) ----
        # tkr: t in (k,r) order
        tkr = sb.tile([P, G, 25], F16, tag="tkr")
        nc.gpsimd.tensor_copy(
            tkr[:].rearrange("p g (k r) -> p g k r", k=5),
            st4[:, :, 64:89].rearrange("p g (r k) -> p g k r", r=5))
        # q2: [p, k, w, r] = a~[w,r] (outer-bcast k) * tkr[k,r] (mid-bcast w)
        q24 = zq.tile([P, G, 800], F16, tag="q24")
        for g in range(G):
            a4v = ae4[:, g, 0:160].rearrange("p (w r) -> p w r", w=H).unsqueeze(1).to_broadcast((P, 5, H, 5))
            t_kr = tkr[:, g, :].rearrange("p (k r) -> p k r", k=5).unsqueeze(2).to_broadcast((P, 5, H, 5))
            eng = nc.gpsimd if g % 2 == 0 else nc.vector
            eng.tensor_tensor(q24[:, g, :].rearrange("p (k w r) -> p k w r", k=5, w=H), a4v, t_kr, OP.mult)
        # ht1 reduce: [p, (g,k,w), r]
        nc.vector.tensor_reduce(
            ws[:, :, WS_HT1: WS_HT1 + 160],
            q24[:].rearrange("p g (c r) -> p (g c) r", r=5),
            axis=AX.X, op=OP.add)

        # G gram: [p, u, v, i]
        qg4 = zq.tile([P, G, 125], F16, tag="qg4")
        for g in range(G):
            t_ui = st4[:, g, 64:89].rearrange("p (u i) -> p u i", u=5).unsqueeze(2).to_broadcast((P, 5, 5, 5))
            t_vi = st4[:, g, 64:89].rearrange("p (v i) -> p v i", v=5).unsqueeze(1).to_broadcast((P, 5, 5, 5))
            eng = nc.gpsimd if g % 2 == 0 else nc.vector
            eng.tensor_tensor(qg4[:, g, :].rearrange("p (u v i) -> p u v i", u=5, v=5), t_ui, t_vi, OP.mult)
        nc.vector.tensor_reduce(
            ws[:, :, WS_G: WS_G + 25],
            qg4[:].rearrange("p g (c i) -> p (g c) i", i=5),
            axis=AX.X, op=OP.add)

        # q7 / M: [p, (k,u), v, j] = E[(k,u),j] (mid-bcast v) * t[v,j] (outer-bcast ku)
        q74 = zq.tile([P, G, 625], F16, tag="q74")
        for g in range(G):
            E3 = ae4[:, g, 160:285].rearrange("p (c j) -> p c j", j=5).unsqueeze(2).to_broadcast((P, 25, 5, 5))
            t_vj = st4[:, g, 64:89].rearrange("p (v j) -> p v j", v=5).unsqueeze(1).to_broadcast((P, 25, 5, 5))
            eng = nc.gpsimd if g % 2 == 0 else nc.vector
            eng.tensor_tensor(q74[:, g, :].rearrange("p (c v j) -> p c v j", c=25, v=5), E3, t_vj, OP.mult)
        nc.vector.tensor_reduce(
            ws[:, :, WS_M: WS_M + 125],
            q74[:].rearrange("p g (c j) -> p (g c) j", j=5),
            axis=AX.X, op=OP.add)

        # ---- XBAR: ws -> feature-major ----
        xb = fm.tile([P, 4 * G, P], F16, tag="xb")
        nc.sync.dma_start_transpose(xb[:], ws[:].rearrange("p g c -> p (g c)"))
        # views: group g, block b at xb[:, 4g+b, :]
        mt_rhs = xb[0:125, :, :].rearrange("f (g b) s -> f g b s", b=4)[:, :, 0, :]
        gt_rhs = xb[0:25, :, :].rearrange("f (g b) s -> f g b s", b=4)[:, :, 1, :]
        h1t = xb[32:64, :, :].rearrange("f (g b) s -> f g b s", b=4)[:, :, 1, :]
        ht1a = xb[:, :, :].rearrange("f (g b) s -> f g b s", b=4)[:, :, 2, :]
        ht1b = xb[0:32, :, :].rearrange("f (g b) s -> f g b s", b=4)[:, :, 3, :]

        # ---- tp2 feature-major ----
        htA_ps = tpsA.tile([P, MS], F32, tag="tpA")
        nc.tensor.matmul(htA_ps[:], W["WtttA"][:], mt_rhs, start=True, stop=True)
        htB_ps = tpsB.tile([32, MS], F32, tag="tpB")
        nc.tensor.matmul(htB_ps[:], W["WtttB"][:], mt_rhs, start=True, stop=True)
        htA = fm.tile([P, MS], F16, tag="htAf")
        nc.vector.tensor_tensor(htA[:].rearrange("f (g s) -> f g s", g=G), htA_ps[:].rearrange("f (g s) -> f g s", g=G), ht1a, OP.add)
        pk96 = fm.tile([96, MS], F16, tag="pk96")
        nc.vector.tensor_tensor(pk96[0:32, :].rearrange("f (g s) -> f g s", g=G), htB_ps[:].rearrange("f (g s) -> f g s", g=G), ht1b, OP.add)

        hs_ps = tpsB.tile([32, MS], F32, tag="tpB")
        nc.tensor.matmul(hs_ps[:], W["Wtts"][:], gt_rhs, start=True, stop=True)
        hs_fm = fm.tile([32, MS], F16, tag="hsf")
        nc.vector.tensor_tensor(hs_fm[:].rearrange("f (g s) -> f g s", g=G), hs_ps[:].rearrange("f (g s) -> f g s", g=G), h1t, OP.add)

        al_ps = tpsB.tile([32, MS], F32, tag="tpB")
        nc.tensor.matmul(al_ps[:], W["M2"][:], hs_fm[:], start=True, stop=True)
        nc.scalar.activation(pk96[64:96, :], al_ps[:], AF.Copy)

        g2A_ps = tpsA.tile([P, MS], F32, tag="tpA")
        nc.tensor.matmul(g2A_ps[:], W["W2A"][:], htA[:], start=True, stop=True)
        g2B_ps = tpsB.tile([32, MS], F32, tag="tpB")
        nc.tensor.matmul(g2B_ps[:], W["W2B"][:], pk96[0:32, :], start=True, stop=True)
        g2A = fm.tile([P, MS], F16, tag="g2Af")
        nc.scalar.activation(g2A[:], g2A_ps[:], AF.Copy)
        nc.scalar.activation(pk96[32:64, :], g2B_ps[:], AF.Copy)

        # ---- XBAR back to sample-major ----
        xhA = fm.tile([P, G, P], F16, tag="xhA")
        nc.sync.dma_start_transpose(xhA[:], htA[:])
        xgA = fm.tile([P, G, P], F16, tag="xgA")
        nc.sync.dma_start_transpose(xgA[:], g2A[:])
        xp = fm.tile([P, G, 96], F16, tag="xp")
        nc.sync.dma_start_transpose(xp[:], pk96[:])

        # contiguous sample-major ht / g2 / alpha
        htb4 = sb.tile([P, G, 160], F16, tag="htb4")
        nc.gpsimd.tensor_copy(htb4[:, :, 0:128], xhA[:])
        nc.gpsimd.tensor_copy(htb4[:, :, 128:160], xp[:, :, 0:32])
        g2b4 = sb.tile([P, G, 160], F16, tag="g2b4")
        nc.gpsimd.tensor_copy(g2b4[:, :, 0:128], xgA[:])
        nc.gpsimd.tensor_copy(g2b4[:, :, 128:160], xp[:, :, 32:64])

        # Q: [p, i, j, v] = g2[i,v] (mid-bcast j) * ht[j,v] (outer-bcast i)
        qq4 = zq.tile([P, G, 800], F16, tag="qq4")
        for g in range(G):
            g2v = g2b4[:, g, :].rearrange("p (i v) -> p i v", i=5).unsqueeze(2).to_broadcast((P, 5, 5, 32))
            htv = htb4[:, g, :].rearrange("p (j v) -> p j v", j=5).unsqueeze(1).to_broadcast((P, 5, 5, 32))
            eng = nc.gpsimd if g % 2 == 0 else nc.vector
            eng.tensor_tensor(qq4[:, g, :].rearrange("p (i j v) -> p i j v", i=5, j=5), g2v, htv, OP.mult)
        Q4 = sb.tile([P, G, 25], F16, tag="Q4")
        qqh = zq.tile([P, G, 25, 16], F16, tag="qqh")
        qv = qq4[:].rearrange("p g (c v) -> p (g c) v", v=32)
        nc.gpsimd.tensor_tensor(
            qqh[:].rearrange("p g c v -> p (g c) v"), qv[:, :, 0:16], qv[:, :, 16:32], OP.add)
        nc.vector.tensor_reduce(
            Q4[:],
            qqh[:].rearrange("p g c v -> p (g c) v"),
            axis=AX.X, op=OP.add)

        # o1: [p, k, c] = Q[c] (outer-bcast k) * CrepC[k,c]
        q10 = zq.tile([P, G, 125], F16, tag="q10")
        for g in range(G):
            Qb = Q4[:, g, :].unsqueeze(1).to_broadcast((P, 5, 25))
            cv = W["CrepC"][:].rearrange("p (k c) -> p k c", k=5)
            eng = nc.gpsimd if g % 2 == 0 else nc.vector
            eng.tensor_tensor(q10[:, g, :].rearrange("p (k c) -> p k c", k=5), Qb, cv, OP.mult)
        # o2: [p, k, v] = alpha[v] (outer-bcast k) * ht[k,v]
        q12 = zq.tile([P, G, 160], F16, tag="q12")
        for g in range(G):
            alb = xp[:, g, 64:96].unsqueeze(1).to_broadcast((P, 5, 32))
            htk = htb4[:, g, :].rearrange("p (k v) -> p k v", k=5)
            eng = nc.gpsimd if g % 2 == 0 else nc.vector
            eng.tensor_tensor(q12[:, g, :].rearrange("p (k v) -> p k v", k=5), alb, htk, OP.mult)
        o14 = sb.tile([P, G, 5], F32, tag="o14")
        nc.vector.tensor_reduce(
            o14[:],
            q10[:].rearrange("p g (c j) -> p (g c) j", j=25),
            axis=AX.X, op=OP.add)
        o24 = sb.tile([P, G, 5], F32, tag="o24")
        nc.vector.tensor_reduce(
            o24[:],
            q12[:].rearrange("p g (c v) -> p (g c) v", v=32),
            axis=AX.X, op=OP.add)
        out4 = io_pool.tile([P, G, 5], F32, tag="out4")
        nc.gpsimd.tensor_add(out4[:], o14[:], o24[:])
        nc.sync.dma_start(out_d[r0: r0 + MS, :].rearrange("(g p) c -> p g c", g=G), out4[:])


def build_program(n_macro=NMACRO):
    nc = bacc.Bacc(
        "TRN2",
        target_bir_lowering=False,
        debug=False,
        enable_asserts=False,
        num_devices=NCORES,
    )
    rows = n_macro * MS
    io = {
        "s": nc.dram_tensor("s", [rows, NS], F16, kind="ExternalInput").ap(),
        "t": nc.dram_tensor("t", [rows, 25], F16, kind="ExternalInput").ap(),
        "stT": nc.dram_tensor("stT", [89, rows], F16, kind="ExternalInput").ap(),
        "out": nc.dram_tensor("out", [rows, 5], F32, kind="ExternalOutput").ap(),
        "w": {
            name: nc.dram_tensor(name, list(shp), F16, kind="ExternalInput").ap()
            for name, shp in WEIGHT_SHAPES.items()
        },
    }
    with tile.TileContext(nc) as tc:
        with ExitStack() as ctx:
            _tile_body(ctx, tc, io, n_macro)
    nc.compile()
    return nc


def make_in_maps(
    scalars, kernel_t2_sum, mc_t2, coulomb_t2, bs_t2, mopac_coulomb_t2,
    w1_sss, w1_stt, w1_tst, w1_tts, w1_ttt, w2_stt, w2_tst, w2_ttt,
):
    wmap = prep_weights(w1_sss, w1_stt, w1_tst, w1_tts, w1_ttt, w2_stt, w2_tst, w2_ttt)
    s = np.ascontiguousarray(np.asarray(scalars, np.float16))
    t = np.stack(
        [
            np.asarray(kernel_t2_sum, np.float32),
            np.asarray(mc_t2, np.float32),
            np.asarray(coulomb_t2, np.float32),
            np.asarray(bs_t2, np.float32),
            np.asarray(mopac_coulomb_t2, np.float32),
        ],
        axis=1,
    ).reshape(B, 25).astype(np.float16)
    in_maps = []
    for c in range(NCORES):
        sh = s[c * BPC: (c + 1) * BPC]
        th = t[c * BPC: (c + 1) * BPC]
        stT = np.concatenate([sh.T, th.T], axis=0)
        m = {"s": sh, "t": np.ascontiguousarray(th), "stT": np.ascontiguousarray(stT)}
        m.update(wmap)
        in_maps.append(m)
    return in_maps


_CACHED_NC = None


def kernel(
    scalars, kernel_t2_sum, mc_t2, coulomb_t2, bs_t2, mopac_coulomb_t2,
    w1_sss, w1_stt, w1_tst, w1_tts, w1_ttt, w2_stt, w2_tst, w2_ttt,
):
    global _CACHED_NC
    if _CACHED_NC is None:
        _CACHED_NC = build_program(NMACRO)
    nc = _CACHED_NC

    in_maps = make_in_maps(
        scalars, kernel_t2_sum, mc_t2, coulomb_t2, bs_t2, mopac_coulomb_t2,
        w1_sss, w1_stt, w1_tst, w1_tts, w1_ttt, w2_stt, w2_tst, w2_ttt,
    )
    res = run_bass_kernel_spmd(nc, in_maps, list(range(NCORES)))
    out = np.concatenate([res.results[c]["out"] for c in range(NCORES)], axis=0)
    return out.astype(np.float32)


# revision 6
# speedup vs baseline: 1.1038x; 1.1038x over previous
# Trainium2 Bass kernel for nn_EquivariantCorrectionHead.
#
# Math (per sample b):
#   s (64,), t (5,5) [u,i]
#   h_s_w = PW1_0*( sum_uv W1sss[u,v,w] s_u s_v + INV_S5 * sum_uv W1tts[u,v,w] G_uv )
#           where G_uv = sum_i t_ui t_vi
#   h_t_wk = PW1_2*( INV_S5*( sum_uv W1stt[u,v,w] s_u t_vk + W1tst tern )
#                    + sum_uv W1ttt[u,v,w] M_kuv ),  M_kuv = sum_ij C_ijk t_ui t_vj
#   out_k = sum_v alpha_v h_t_vk + sum_ij C_ijk Q_ij
#           alpha_v = sum_u (PW2_2*INV_S5)*(W2stt[u,v]+W2tst[v,u]) h_s_u
#           Q_ij = sum_v g2_vi h_t_vj, g2_vi = PW2_2 * sum_u W2ttt[u,v] h_t_ui
#
# Device mapping per 128-sample tile:
#   PE: s@Wbig (quadratic-form left factor + stt/tst columns), C-contraction (E),
#       block-diag ttt map, block-diag g2 map, alpha map, all transposes.
#   DVE: per-sample bilinears as broadcast-AP multiply + innermost-axis reduce.
#   ACT: psum->sbuf copies.
# Data parallel over 8 cores (batch sharded, weights replicated).

import os
import sys
from contextlib import ExitStack

import numpy as np

if "/opt/trn_rl_repo" not in sys.path:
    sys.path.insert(0, "/opt/trn_rl_repo")

import concourse.bass as bass
import concourse.mybir as mybir
import concourse.tile as tile
from concourse import bacc, masks
from concourse.bass_utils import run_bass_kernel_spmd

B, NS, H = 131072, 64, 32
NCORES = 8
BPC = B // NCORES          # 16384 rows per core
P = 128                    # samples per tile
NT_FULL = BPC // P         # 128 tiles per core

PW1_0 = float((NS * NS + 25.0) ** -0.5)
PW1_2 = float((5.0 / (10.0 * NS + 25.0)) ** 0.5)
PW2_2 = float((5.0 / (3.0 * H * H)) ** 0.5)
INV_S5 = float(5.0 ** -0.5)

F32 = mybir.dt.float32
F16 = mybir.dt.float16
AX = mybir.AxisListType
OP = mybir.AluOpType


def _wigner3j_222():
    s2, s6 = np.sqrt(2.0), np.sqrt(6.0)
    M = np.zeros((5, 3, 3))
    M[0] = np.array([[0.0, 1, 0], [1, 0, 0], [0, 0, 0]]) / s2
    M[1] = np.array([[0.0, 0, 0], [0, 0, 1], [0, 1, 0]]) / s2
    M[2] = np.diag([-1.0, -1, 2]) / s6
    M[3] = np.array([[0.0, 0, 1], [0, 0, 0], [1, 0, 0]]) / s2
    M[4] = np.diag([1.0, -1, 0]) / s2
    C = np.einsum("aij,bjk,cki->abc", M, M, M)
    C = 0.5 * (C + C.transpose(1, 0, 2))
    return (C / np.linalg.norm(C)).astype(np.float64)


def prep_weights(w1_sss, w1_stt, w1_tst, w1_tts, w1_ttt, w2_stt, w2_tst, w2_ttt):
    """Host-side weight preprocessing. Returns dict of device const arrays (f32)."""
    C = _wigner3j_222()
    w1_sss = np.asarray(w1_sss, np.float64)
    w1_stt = np.asarray(w1_stt, np.float64)
    w1_tst = np.asarray(w1_tst, np.float64)
    w1_tts = np.asarray(w1_tts, np.float64)
    w1_ttt = np.asarray(w1_ttt, np.float64)
    w2_stt = np.asarray(w2_stt, np.float64)[:, :, 0]
    w2_tst = np.asarray(w2_tst, np.float64)[:, :, 0]
    w2_ttt = np.asarray(w2_ttt, np.float64)[:, :, 0]

    # Wbig [64, 2048 + 160]: cols[w*64+v] = PW1_0*W1sss[u,v,w];
    # cols[2048 + w*5+r] = PW1_2*INV_S5*(W1stt[u,r,w] + W1tst[r,u,w])
    wb_sss = PW1_0 * np.transpose(w1_sss, (0, 2, 1)).reshape(NS, H * NS)  # u,(w,v)
    wb_ad = (PW1_2 * INV_S5) * (
        np.transpose(w1_stt, (0, 2, 1)) + np.transpose(w1_tst, (1, 2, 0))
    ).reshape(NS, H * 5)  # u,(w,r)
    Wbig = np.concatenate([wb_sss, wb_ad], axis=1)  # [64, 2208]

    # Wtts matmul lhsT [(u,v)=25, w=32]
    Wtts_mat = (PW1_0 * INV_S5) * w1_tts.reshape(25, H)

    # E-step lhsT: Cbig [(u',i)=25, (u,j,k)=125], Cbig[u'*5+i, u*25+j*5+k] = d_{u'u} C[i,j,k]
    # Cbig[(u',i), (k,u,j)] = d_{u'u} C[i,j,k]
    Cbig = np.zeros((25, 125))
    for u in range(5):
        for j in range(5):
            for k in range(5):
                Cbig[u * 5 : u * 5 + 5, k * 25 + u * 5 + j] = C[:, j, k]
    # ttt block-diag lhsT: [(k,u,v)=125, (k',w)]: d_{kk'} * PW1_2*W1ttt[u,v,w]
    wttt = PW1_2 * w1_ttt.reshape(25, H)  # (u,v),w
    WtttA = np.zeros((125, 128))  # k'=0..3
    WtttB = np.zeros((125, 32))   # k'=4
    for k in range(4):
        WtttA[k * 25 : k * 25 + 25, k * 32 : k * 32 + 32] = wttt
    WtttB[100:125, :] = wttt

    # g2 block-diag lhsT: [(i,u), (i',v)]: d_{ii'} * PW2_2*W2ttt[u,v]
    w2t = PW2_2 * w2_ttt
    W2A = np.zeros((128, 128))  # i=0..3
    for i in range(4):
        W2A[i * 32 : i * 32 + 32, i * 32 : i * 32 + 32] = w2t
    W2B = w2t.copy()  # i=4, [32,32]

    # alpha map lhsT M2 [u,v]
    M2 = (PW2_2 * INV_S5) * (w2_stt + w2_tst.T)

    # C replicated for final contraction: [128, (k,ij)=125], val C[i,j,k]
    crep = np.transpose(C, (2, 0, 1)).reshape(1, 125)
    Crep2 = np.broadcast_to(crep, (P, 125)).copy()

    return {
        "Wbig": np.ascontiguousarray(Wbig, np.float16),
        "Wtts": np.ascontiguousarray(Wtts_mat, np.float16),
        "Cbig": np.ascontiguousarray(Cbig, np.float16),
        "WtttA": np.ascontiguousarray(WtttA, np.float16),
        "WtttB": np.ascontiguousarray(WtttB, np.float16),
        "W2A": np.ascontiguousarray(W2A, np.float16),
        "W2B": np.ascontiguousarray(W2B, np.float16),
        "M2": np.ascontiguousarray(M2, np.float16),
        "Crep2": np.ascontiguousarray(Crep2, np.float16),
    }


WEIGHT_SHAPES = {
    "Wbig": (NS, H * NS + H * 5),
    "Wtts": (25, H),
    "Cbig": (25, 125),
    "WtttA": (125, 128),
    "WtttB": (125, 32),
    "W2A": (128, 128),
    "W2B": (32, 32),
    "M2": (32, 32),
    "Crep2": (P, 125),
}


def _tile_body(ctx: ExitStack, tc: tile.TileContext, io, n_tiles: int):
    nc = tc.nc
    s_d, t_d, out_d, wd = io["s"], io["t"], io["out"], io["w"]
    sT_d, tT_d = io["sT"], io["tT"]

    const = ctx.enter_context(tc.tile_pool(name="const", bufs=1))
    # Load weight consts into sbuf once
    W = {}
    for name, shp in WEIGHT_SHAPES.items():
        W[name] = const.tile(list(shp), F16, tag=name, name=f"W_{name}")
        nc.sync.dma_start(W[name][:], wd[name])
    ident = const.tile([128, 128], F16, tag="ident")
    masks.make_identity(nc, ident[:])
    ident32 = const.tile([128, 128], F32, tag="ident32")
    masks.make_identity(nc, ident32[:])

    # sbuf pools
    io_pool = ctx.enter_context(tc.tile_pool(name="io", bufs=3))
    sb = ctx.enter_context(tc.tile_pool(name="sb", bufs=2))
    qb = ctx.enter_context(tc.tile_pool(name="qb", bufs=2))
    # psum pools
    zps = ctx.enter_context(tc.tile_pool(name="zps", bufs=2, space="PSUM"))
    aps_ = ctx.enter_context(tc.tile_pool(name="aps", bufs=1, space="PSUM"))
    tps = ctx.enter_context(tc.tile_pool(name="tps", bufs=2, space="PSUM"))

    NW = 4  # sss psum chunks (8 w's each)

    ctx.enter_context(nc.allow_low_precision("fp16 intermediates fit the 2e-2 budget"))
    for it in range(n_tiles):
        r0 = it * P
        # --- load inputs ---
        st = io_pool.tile([P, 96], F16, tag="st")
        nc.sync.dma_start(st[:, 0:64], s_d[r0 : r0 + P, :])
        nc.sync.dma_start(st[:, 64:89], t_d[r0 : r0 + P, :])
        s_sb = st[:, 0:64]
        t_sb = st[:, 64:89]

        # --- feature-major copies come pre-transposed from the host ---
        sT = sb.tile([64, P], F16, tag="sT")
        nc.sync.dma_start(sT[:], sT_d[:, r0 : r0 + P])
        tT = sb.tile([25, P], F16, tag="tT")
        nc.sync.dma_start(tT[:], tT_d[:, r0 : r0 + P])

        # --- big matmul: z chunks (sss) + a~ (stt/tst) ---
        h1 = sb.tile([P, H], F32, tag="h1")
        for wc in range(NW):
            zc = zps.tile([P, 512], F32, tag="zc")
            nc.tensor.matmul(
                zc[:], sT[:], W["Wbig"][:, wc * 512 : wc * 512 + 512],
                start=True, stop=True,
            )
            # cast z chunk to f16 on ACT, then q1 = z * s and reduce over v on DVE
            zh = qb.tile([P, 512], F16, tag="zh")
            nc.scalar.copy(zh[:], zc[:])
            q1 = qb.tile([P, 512], F16, tag="q1")
            z3 = zh[:].rearrange("p (w v) -> p w v", w=8)
            s3 = s_sb.unsqueeze(1).to_broadcast((P, 8, 64))
            nc.vector.tensor_tensor(q1[:].rearrange("p (w v) -> p w v", w=8), z3, s3, OP.mult)
            nc.vector.tensor_reduce(
                h1[:, wc * 8 : wc * 8 + 8], q1[:].rearrange("p (w v) -> p w v", w=8),
                axis=AX.X, op=OP.add,
            )

        a_ps = aps_.tile([P, H * 5], F32, tag="a")
        nc.tensor.matmul(a_ps[:], sT[:], W["Wbig"][:, 2048:2208], start=True, stop=True)
        # ht1[k,w] = sum_r a~[w,r] * t[r,k]
        ah = sb.tile([P, H * 5], F16, tag="ah")
        nc.scalar.copy(ah[:], a_ps[:])
        tkr = sb.tile([P, 25], F16, tag="tkr")
        nc.gpsimd.tensor_copy(tkr[:].rearrange("p (k r) -> p k r", k=5),
                              t_sb.rearrange("p (r k) -> p k r", r=5))
        q2 = qb.tile([P, 5 * H * 5], F16, tag="q2")
        q2v = q2[:].rearrange("p (k w r) -> p k w r", k=5, w=H)
        a4 = ah[:].rearrange("p (w r) -> p w r", w=H).unsqueeze(1).to_broadcast((P, 5, H, 5))
        t_kr = tkr[:].rearrange("p (k r) -> p k r", k=5).unsqueeze(2).to_broadcast((P, 5, H, 5))
        nc.vector.tensor_tensor(q2v, a4, t_kr, OP.mult)
        ht1 = sb.tile([P, 5 * H], F32, tag="ht1")
        nc.vector.tensor_reduce(ht1[:].rearrange("p (k w) -> p k w", k=5), q2v, axis=AX.X, op=OP.add)

        # --- gram G[u,v] = sum_i t_ui t_vi ---
        qg = qb.tile([P, 125], F16, tag="qg")
        qgv = qg[:].rearrange("p (u v i) -> p u v i", u=5, v=5)
        t_ui = t_sb.rearrange("p (u i) -> p u i", u=5).unsqueeze(2).to_broadcast((P, 5, 5, 5))
        t_vi = t_sb.rearrange("p (v i) -> p v i", v=5).unsqueeze(1).to_broadcast((P, 5, 5, 5))
        nc.gpsimd.tensor_tensor(qgv, t_ui, t_vi, OP.mult)
        G = sb.tile([P, 25], F16, tag="G")
        nc.vector.tensor_reduce(G[:].rearrange("p (u v) -> p u v", u=5), qgv, axis=AX.X, op=OP.add)

        # --- tts via PE: hsT = Wtts_mat^T @ G^T + transpose(h1), fused in psum ---
        GT_ps = tps.tile([25, P], F16, tag="tp")
        nc.tensor.transpose(GT_ps[:], G[:], ident[:])
        GT = sb.tile([25, P], F16, tag="GT")
        nc.scalar.copy(GT[:], GT_ps[:])

        # --- E sample-major directly: E[b,(u,j,k)] = sum_(ui) t[b,(ui)] Cbig ---
        Eb_ps = tps.tile([P, 125], F32, tag="tp")
        nc.tensor.matmul(Eb_ps[:], tT[:], W["Cbig"][:], start=True, stop=True)

        Eh = sb.tile([P, 125], F16, tag="Eh")
        nc.scalar.copy(Eh[:], Eb_ps[:])

        # --- M[(k,u,v)] = sum_j E[u,j,k] t[v,j]  (5 ops: ISA max 3 free dims) ---
        q7 = qb.tile([P, 625], F16, tag="q7")
        t_vj = (
            t_sb.rearrange("p (v j) -> p v j", v=5)
            .unsqueeze(1)
            .to_broadcast((P, 5, 5, 5))
        )
        M = sb.tile([P, 125], F16, tag="M")
        for k in range(5):
            q7k = q7[:, k * 125 : (k + 1) * 125].rearrange("p (u v j) -> p u v j", u=5, v=5)
            E3 = Eh[:].rearrange("p (k u j) -> p k u j", k=5, u=5)[:, k].unsqueeze(
                2
            ).to_broadcast((P, 5, 5, 5))
            nc.gpsimd.tensor_tensor(q7k, E3, t_vj, OP.mult)
        nc.vector.tensor_reduce(
            M[:].rearrange("p (c j) -> p c j", j=5), q7[:].rearrange("p (c j) -> p c j", j=5),
            axis=AX.X, op=OP.add,
        )

        # --- ht2T = blockdiag(W1ttt) @ M^T ; fuse with ht1T ---
        MT_ps = tps.tile([125, P], F16, tag="tp")
        nc.tensor.transpose(MT_ps[:], M[:], ident[:])
        MT = sb.tile([125, P], F16, tag="MT")
        nc.scalar.copy(MT[:], MT_ps[:])

        htTA_ps = tps.tile([P, P], F32, tag="tpA")
        htTB_ps = tps.tile([32, P], F32, tag="tpB", bufs=1)
        nc.tensor.matmul(htTA_ps[:], W["WtttA"][:], MT[:], start=True, stop=False)
        nc.tensor.matmul(htTB_ps[:], W["WtttB"][:], MT[:], start=True, stop=False)
        # add ht1T into the same psum accumulation via transpose (PE accumulates)
        nc.tensor.matmul(htTA_ps[:], ht1[:, 0:128], ident32[:], is_transpose=True, start=False, stop=True)
        nc.tensor.matmul(htTB_ps[:], ht1[:, 128:160], ident32[:], is_transpose=True, start=False, stop=True)
        htTA = sb.tile([P, P], F16, tag="htTA")
        nc.scalar.copy(htTA[:], htTA_ps[:])
        htTB = sb.tile([32, P], F16, tag="htTB")
        nc.scalar.copy(htTB[:], htTB_ps[:])

        # h_t in sample-major layout: [p, (k,w)] -- both transposes into one tile
        htb_ps = tps.tile([P, 160], F16, tag="tpA")
        nc.tensor.transpose(htb_ps[:, 0:128], htTA[:], ident[:])
        nc.tensor.transpose(htb_ps[:, 128:160], htTB[:], ident[0:32, 0:32])
        htb = sb.tile([P, 160], F16, tag="htb")
        nc.scalar.copy(htb[:], htb_ps[:])

        # --- g2T = blockdiag(W2ttt) @ htT ---
        g2A_ps = tps.tile([P, P], F32, tag="tpA")
        nc.tensor.matmul(g2A_ps[:], W["W2A"][:], htTA[:], start=True, stop=True)
        g2B_ps = tps.tile([32, P], F32, tag="tpB", bufs=1)
        nc.tensor.matmul(g2B_ps[:], W["W2B"][:], htTB[:], start=True, stop=True)
        g2A_sb = sb.tile([P, P], F16, tag="g2A")
        nc.scalar.copy(g2A_sb[:], g2A_ps[:])
        g2B_sb = sb.tile([32, P], F16, tag="g2B")
        nc.scalar.copy(g2B_sb[:], g2B_ps[:])
        g2b_ps = tps.tile([P, 160], F16, tag="tpA")    # [p, (i,v)]
        nc.tensor.transpose(g2b_ps[:, 0:128], g2A_sb[:], ident[:])
        nc.tensor.transpose(g2b_ps[:, 128:160], g2B_sb[:], ident[0:32, 0:32])

        g2h = sb.tile([P, 160], F16, tag="g2h")
        nc.scalar.copy(g2h[:], g2b_ps[:])

        # --- Q[i,j] = sum_v g2[(i,v)] ht[(j,v)] ---
        Q = sb.tile([P, 25], F16, tag="Q")
        qq = qb.tile([P, 800], F16, tag="qq")
        qqv = qq[:].rearrange("p (i j v) -> p i j v", i=5, j=5)
        g2_b = g2h[:].rearrange("p (i v) -> p i v", i=5).unsqueeze(2).to_broadcast((P, 5, 5, 32))
        ht_b = htb[:].rearrange("p (j v) -> p j v", j=5).unsqueeze(1).to_broadcast((P, 5, 5, 32))
        nc.vector.tensor_tensor(qqv, g2_b, ht_b, OP.mult)
        nc.vector.tensor_reduce(
            Q[:].rearrange("p (i j) -> p i j", i=5), qqv, axis=AX.X, op=OP.add
        )

        # --- o1[k] = sum_ij C[i,j,k] Q[i,j] ---
        q10 = qb.tile([P, 125], F16, tag="q10")
        q10v = q10[:].rearrange("p (k c) -> p k c", k=5)
        Q_b = Q[:].unsqueeze(1).to_broadcast((P, 5, 25))
        crep_v = W["Crep2"][:].rearrange("p (k c) -> p k c", k=5)
        nc.gpsimd.tensor_tensor(q10v, Q_b, crep_v, OP.mult)
        o1 = sb.tile([P, 5], F16, tag="o1")
        nc.vector.tensor_reduce(o1[:], q10v, axis=AX.X, op=OP.add)

        # --- alpha = M2^T @ h_s (per-sample), then o2[k] = sum_v alpha_v ht[(k,v)] ---
        hsT_ps = tps.tile([H, P], F32, tag="tp")
        nc.tensor.matmul(hsT_ps[:], W["Wtts"][:], GT[:], start=True, stop=False)
        nc.tensor.matmul(hsT_ps[:], h1[:], ident32[:], is_transpose=True, start=False, stop=True)
        hsT = sb.tile([H, P], F16, tag="hsT")
        nc.scalar.copy(hsT[:], hsT_ps[:])
        alT_ps = tps.tile([H, P], F32, tag="tp")
        nc.tensor.matmul(alT_ps[:], W["M2"][:], hsT[:], start=True, stop=True)
        alT = sb.tile([H, P], F16, tag="alT")
        nc.scalar.copy(alT[:], alT_ps[:])
        al_ps = tps.tile([P, H], F16, tag="tp")
        nc.tensor.transpose(al_ps[:], alT[:], ident[0:32, 0:32])

        alh = sb.tile([P, H], F16, tag="alh")
        nc.scalar.copy(alh[:], al_ps[:])
        q12 = qb.tile([P, 160], F16, tag="q12")
        q12v = q12[:].rearrange("p (k v) -> p k v", k=5)
        al_b = alh[:].unsqueeze(1).to_broadcast((P, 5, 32))
        ht_kv = htb[:].rearrange("p (k v) -> p k v", k=5)
        nc.vector.tensor_tensor(q12v, al_b, ht_kv, OP.mult)
        o2 = sb.tile([P, 5], F16, tag="o2")
        nc.vector.tensor_reduce(o2[:], q12v, axis=AX.X, op=OP.add)

        out_sb = io_pool.tile([P, 5], F32, tag="out_sb")
        nc.gpsimd.tensor_add(out_sb[:], o1[:], o2[:])
        nc.sync.dma_start(out_d[r0 : r0 + P, :], out_sb[:])


def build_program(n_tiles=NT_FULL):
    nc = bacc.Bacc(
        "TRN2",
        target_bir_lowering=False,
        debug=False,
        enable_asserts=False,
        num_devices=NCORES,
    )
    rows = n_tiles * P
    io = {
        "s": nc.dram_tensor("s", [rows, NS], F16, kind="ExternalInput").ap(),
        "t": nc.dram_tensor("t", [rows, 25], F16, kind="ExternalInput").ap(),
        "sT": nc.dram_tensor("sT", [NS, rows], F16, kind="ExternalInput").ap(),
        "tT": nc.dram_tensor("tT", [25, rows], F16, kind="ExternalInput").ap(),
        "out": nc.dram_tensor("out", [rows, 5], F32, kind="ExternalOutput").ap(),
        "w": {
            name: nc.dram_tensor(name, list(shp), F16, kind="ExternalInput").ap()
            for name, shp in WEIGHT_SHAPES.items()
        },
    }
    with tile.TileContext(nc) as tc:
        with ExitStack() as ctx:
            _tile_body(ctx, tc, io, n_tiles)
    nc.compile()
    return nc




def make_in_maps(
    scalars, kernel_t2_sum, mc_t2, coulomb_t2, bs_t2, mopac_coulomb_t2,
    w1_sss, w1_stt, w1_tst, w1_tts, w1_ttt, w2_stt, w2_tst, w2_ttt,
):
    wmap = prep_weights(w1_sss, w1_stt, w1_tst, w1_tts, w1_ttt, w2_stt, w2_tst, w2_ttt)
    s = np.ascontiguousarray(np.asarray(scalars, np.float16))
    t = np.stack(
        [
            np.asarray(kernel_t2_sum, np.float32),
            np.asarray(mc_t2, np.float32),
            np.asarray(coulomb_t2, np.float32),
            np.asarray(bs_t2, np.float32),
            np.asarray(mopac_coulomb_t2, np.float32),
        ],
        axis=1,
    ).reshape(B, 25)
    t = np.ascontiguousarray(t.astype(np.float16))
    in_maps = []
    for c in range(NCORES):
        sh = s[c * BPC : (c + 1) * BPC]
        th = t[c * BPC : (c + 1) * BPC]
        m = {
            "s": sh,
            "t": th,
            "sT": np.ascontiguousarray(sh.T),
            "tT": np.ascontiguousarray(th.T),
        }
        m.update(wmap)
        in_maps.append(m)
    return in_maps

_CACHED_NC = None


def kernel(
    scalars, kernel_t2_sum, mc_t2, coulomb_t2, bs_t2, mopac_coulomb_t2,
    w1_sss, w1_stt, w1_tst, w1_tts, w1_ttt, w2_stt, w2_tst, w2_ttt,
):
    global _CACHED_NC
    if _CACHED_NC is None:
        _CACHED_NC = build_program(NT_FULL)
    nc = _CACHED_NC

    in_maps = make_in_maps(
        scalars, kernel_t2_sum, mc_t2, coulomb_t2, bs_t2, mopac_coulomb_t2,
        w1_sss, w1_stt, w1_tst, w1_tts, w1_ttt, w2_stt, w2_tst, w2_ttt,
    )
    res = run_bass_kernel_spmd(nc, in_maps, list(range(NCORES)))
    out = np.concatenate([res.results[c]["out"] for c in range(NCORES)], axis=0)
    return out.astype(np.float32)



# revision 8
# speedup vs baseline: 1.2244x; 1.1093x over previous
# Trainium2 Bass kernel for nn_EquivariantCorrectionHead — v3.
# All-eigen sss (ACT squares PSUM), Pool tree-L1 + DVE finish reduces,
# XBAR transposes for tp2, 4-group macro tiles.

import sys
from contextlib import ExitStack

import numpy as np

if "/opt/trn_rl_repo" not in sys.path:
    sys.path.insert(0, "/opt/trn_rl_repo")

import concourse.bass as bass
import concourse.mybir as mybir
import concourse.tile as tile
from concourse import bacc
from concourse.bass_utils import run_bass_kernel_spmd

B, NS, H = 131072, 64, 32
NCORES = 8
BPC = B // NCORES
P = 128
G = 4
MS = P * G
NMACRO = BPC // MS

PW1_0 = float((NS * NS + 25.0) ** -0.5)
PW1_2 = float((5.0 / (10.0 * NS + 25.0)) ** 0.5)
PW2_2 = float((5.0 / (3.0 * H * H)) ** 0.5)
INV_S5 = float(5.0 ** -0.5)

F32 = mybir.dt.float32
F16 = mybir.dt.float16
AX = mybir.AxisListType
OP = mybir.AluOpType
AF = mybir.ActivationFunctionType

CE = 64 + 32 * 64          # 2112: id + 32 eigen w's
CAE = 160 + 125
NCOL = CE + CAE            # 2397


def _wigner3j_222():
    s2, s6 = np.sqrt(2.0), np.sqrt(6.0)
    M = np.zeros((5, 3, 3))
    M[0] = np.array([[0.0, 1, 0], [1, 0, 0], [0, 0, 0]]) / s2
    M[1] = np.array([[0.0, 0, 0], [0, 0, 1], [0, 1, 0]]) / s2
    M[2] = np.diag([-1.0, -1, 2]) / s6
    M[3] = np.array([[0.0, 0, 1], [0, 0, 0], [1, 0, 0]]) / s2
    M[4] = np.diag([1.0, -1, 0]) / s2
    C = np.einsum("aij,bjk,cki->abc", M, M, M)
    C = 0.5 * (C + C.transpose(1, 0, 2))
    return C / np.linalg.norm(C)


def prep_weights(w1_sss, w1_stt, w1_tst, w1_tts, w1_ttt, w2_stt, w2_tst, w2_ttt):
    C = _wigner3j_222()
    w1_sss = np.asarray(w1_sss, np.float64)
    w1_stt = np.asarray(w1_stt, np.float64)
    w1_tst = np.asarray(w1_tst, np.float64)
    w1_tts = np.asarray(w1_tts, np.float64)
    w1_ttt = np.asarray(w1_ttt, np.float64)
    w2_stt = np.asarray(w2_stt, np.float64)[:, :, 0]
    w2_tst = np.asarray(w2_tst, np.float64)[:, :, 0]
    w2_ttt = np.asarray(w2_ttt, np.float64)[:, :, 0]

    Wall = np.zeros((89, NCOL))
    cvec = np.zeros(32)
    Wall[0:64, 0:64] = np.eye(64)
    for w in range(32):
        A = PW1_0 * w1_sss[:, :, w]
        S = 0.5 * (A + A.T)
        lam, Q = np.linalg.eigh(S)
        c = max(0.0, -lam.min()) + 1e-6
        Wall[0:64, 64 + w * 64: 64 + (w + 1) * 64] = Q * np.sqrt(lam + c)
        cvec[w] = c
    wb_ad = (PW1_2 * INV_S5) * (
        np.transpose(w1_stt, (0, 2, 1)) + np.transpose(w1_tst, (1, 2, 0))
    ).reshape(NS, H * 5)
    Wall[0:64, CE: CE + 160] = wb_ad
    Cbig = np.zeros((25, 125))
    for u in range(5):
        for j in range(5):
            for k in range(5):
                Cbig[u * 5: u * 5 + 5, k * 25 + u * 5 + j] = C[:, j, k]
    Wall[64:89, CE + 160: NCOL] = Cbig

    Wtts = (PW1_0 * INV_S5) * w1_tts.reshape(25, H)
    wttt = PW1_2 * w1_ttt.reshape(25, H)
    WtttA = np.zeros((125, 128))
    WtttB = np.zeros((125, 32))
    for k in range(4):
        WtttA[k * 25: k * 25 + 25, k * 32: k * 32 + 32] = wttt
    WtttB[100:125, :] = wttt
    w2t = PW2_2 * w2_ttt
    W2A = np.zeros((128, 128))
    for i in range(4):
        W2A[i * 32: i * 32 + 32, i * 32: i * 32 + 32] = w2t
    W2B = w2t.copy()
    M2 = (PW2_2 * INV_S5) * (w2_stt + w2_tst.T)
    crepC = np.broadcast_to(np.transpose(C, (2, 0, 1)).reshape(1, 125), (P, 125)).copy()
    crep_cw = np.broadcast_to(cvec.reshape(1, 32), (P, 32)).copy()

    f16 = lambda x: np.ascontiguousarray(x, np.float16)
    return {
        "Wall": f16(Wall), "Wtts": f16(Wtts), "WtttA": f16(WtttA),
        "WtttB": f16(WtttB), "W2A": f16(W2A), "W2B": f16(W2B),
        "M2": f16(M2), "CrepC": f16(crepC), "CrepW": f16(crep_cw),
    }


WEIGHT_SHAPES = {
    "Wall": (89, NCOL), "Wtts": (25, H), "WtttA": (125, 128),
    "WtttB": (125, 32), "W2A": (128, 128), "W2B": (32, 32),
    "M2": (32, 32), "CrepC": (P, 125), "CrepW": (P, 32),
}

WS_M = 0
WS_G = 128
WS_H1 = 160
WS_HT1 = 256


def _tile_body(ctx: ExitStack, tc: tile.TileContext, io, n_macro: int):
    nc = tc.nc
    s_d, t_d, stT_d, out_d, wd = io["s"], io["t"], io["stT"], io["out"], io["w"]

    const = ctx.enter_context(tc.tile_pool(name="const", bufs=1))
    W = {}
    for name, shp in WEIGHT_SHAPES.items():
        W[name] = const.tile(list(shp), F16, tag=name, name=f"W_{name}")
        nc.sync.dma_start(W[name][:], wd[name])

    io_pool = ctx.enter_context(tc.tile_pool(name="io", bufs=2))
    zq = ctx.enter_context(tc.tile_pool(name="zq", bufs=2))
    sb = ctx.enter_context(tc.tile_pool(name="sb", bufs=2))
    fm = ctx.enter_context(tc.tile_pool(name="fm", bufs=2))
    zps = ctx.enter_context(tc.tile_pool(name="zps", bufs=1, space="PSUM"))
    tpsA = ctx.enter_context(tc.tile_pool(name="tpsA", bufs=2, space="PSUM"))
    tpsB = ctx.enter_context(tc.tile_pool(name="tpsB", bufs=1, space="PSUM"))

    ctx.enter_context(nc.allow_low_precision("fp16 intermediates fit the 2e-2 budget"))

    for im in range(n_macro):
        r0 = im * MS
        st4 = io_pool.tile([P, G, 96], F16, tag="st4")
        nc.sync.dma_start(st4[:, :, 0:64], s_d[r0: r0 + MS, :].rearrange("(g p) u -> p g u", g=G))
        nc.sync.dma_start(st4[:, :, 64:89], t_d[r0: r0 + MS, :].rearrange("(g p) u -> p g u", g=G))
        stT4 = io_pool.tile([89, MS], F16, tag="stT4")
        nc.sync.dma_start(stT4[:], stT_d[:, r0: r0 + MS])

        zsqE = zq.tile([P, G, CE], F16, tag="zsqE")
        ae4 = sb.tile([P, G, 285], F16, tag="ae4")

        for g in range(G):
            lhs = stT4[:, g * P: (g + 1) * P]
            zp = zps.tile([P, NCOL], F32, tag="zp")
            for c0 in range(0, 2048, 512):
                nc.tensor.matmul(zp[:, c0: c0 + 512], lhs, W["Wall"][:, c0: c0 + 512], start=True, stop=True)
            nc.tensor.matmul(zp[:, 2048:NCOL], lhs, W["Wall"][:, 2048:NCOL], start=True, stop=True)
            # ACT squares everything (id + 32 eigen w's)
            nc.scalar.activation(zsqE[:, g, 0:1024], zp[:, 0:1024], AF.Square)
            nc.scalar.activation(zsqE[:, g, 1024:CE], zp[:, 1024:CE], AF.Square)
            nc.scalar.activation(ae4[:, g, :], zp[:, CE:NCOL], AF.Copy)

        # ---- h1 ----
        tmp_e = sb.tile([P, G, 33], F32, tag="tmp_e")
        zhE = zq.tile([P, G, 33, 32], F16, tag="zhE")
        zE4 = zsqE[:].rearrange("p g (w h v) -> p (g w) h v", h=2, v=32)
        nc.gpsimd.tensor_tensor(
            zhE[:].rearrange("p g w v -> p (g w) v"), zE4[:, :, 0, :], zE4[:, :, 1, :], OP.add)
        nc.vector.tensor_reduce(
            tmp_e[:], zhE[:].rearrange("p g w v -> p (g w) v"), axis=AX.X, op=OP.add)

        ws = sb.tile([P, G, 512], F16, tag="ws")
        tmp2 = sb.tile([P, G, 32], F32, tag="tmp2")
        s2b = tmp_e[:, :, 0:1].to_broadcast((P, G, 32))
        cwb = W["CrepW"][:].unsqueeze(1).to_broadcast((P, G, 32))
        nc.vector.tensor_tensor(tmp2[:], s2b, cwb, OP.mult)
        nc.vector.tensor_tensor(ws[:, :, WS_H1: WS_H1 + 32], tmp_e[:, :, 1:33], tmp2[:], OP.subtract)

        # ---- small bilinears ----
        tkr = sb.tile([P, G, 25], F16, tag="tkr")
        nc.gpsimd.tensor_copy(
            tkr[:].rearrange("p g (k r) -> p g k r", k=5),
            st4[:, :, 64:89].rearrange("p g (r k) -> p g k r", r=5))
        q24 = zq.tile([P, G, 800], F16, tag="q24")
        for g in range(G):
            a4v = ae4[:, g, 0:160].rearrange("p (w r) -> p w r", w=H).unsqueeze(1).to_broadcast((P, 5, H, 5))
            t_kr = tkr[:, g, :].rearrange("p (k r) -> p k r", k=5).unsqueeze(2).to_broadcast((P, 5, H, 5))
            eng = nc.gpsimd if g < 3 else nc.vector
            eng.tensor_tensor(q24[:, g, :].rearrange("p (k w r) -> p k w r", k=5, w=H), a4v, t_kr, OP.mult)
        nc.vector.tensor_reduce(
            ws[:, :, WS_HT1: WS_HT1 + 160],
            q24[:].rearrange("p g (c r) -> p (g c) r", r=5), axis=AX.X, op=OP.add)

        qg4 = zq.tile([P, G, 125], F16, tag="qg4")
        for g in range(G):
            t_ui = st4[:, g, 64:89].rearrange("p (u i) -> p u i", u=5).unsqueeze(2).to_broadcast((P, 5, 5, 5))
            t_vi = st4[:, g, 64:89].rearrange("p (v i) -> p v i", v=5).unsqueeze(1).to_broadcast((P, 5, 5, 5))
            eng = nc.gpsimd if g < 3 else nc.vector
            eng.tensor_tensor(qg4[:, g, :].rearrange("p (u v i) -> p u v i", u=5, v=5), t_ui, t_vi, OP.mult)
        nc.vector.tensor_reduce(
            ws[:, :, WS_G: WS_G + 25],
            qg4[:].rearrange("p g (c i) -> p (g c) i", i=5), axis=AX.X, op=OP.add)

        q74 = zq.tile([P, G, 625], F16, tag="q74")
        for g in range(G):
            E3 = ae4[:, g, 160:285].rearrange("p (c j) -> p c j", j=5).unsqueeze(2).to_broadcast((P, 25, 5, 5))
            t_vj = st4[:, g, 64:89].rearrange("p (v j) -> p v j", v=5).unsqueeze(1).to_broadcast((P, 25, 5, 5))
            eng = nc.gpsimd if g < 3 else nc.vector
            eng.tensor_tensor(q74[:, g, :].rearrange("p (c v j) -> p c v j", c=25, v=5), E3, t_vj, OP.mult)
        nc.vector.tensor_reduce(
            ws[:, :, WS_M: WS_M + 125],
            q74[:].rearrange("p g (c j) -> p (g c) j", j=5), axis=AX.X, op=OP.add)

        # ---- XBAR forward ----
        xb = fm.tile([P, 4 * G, P], F16, tag="xb")
        nc.sync.dma_start_transpose(xb[:], ws[:].rearrange("p g c -> p (g c)"))
        mt_rhs = xb[0:125, :, :].rearrange("f (g b) s -> f g b s", b=4)[:, :, 0, :]
        gt_rhs = xb[0:25, :, :].rearrange("f (g b) s -> f g b s", b=4)[:, :, 1, :]
        h1t = xb[32:64, :, :].rearrange("f (g b) s -> f g b s", b=4)[:, :, 1, :]
        ht1a = xb[:, :, :].rearrange("f (g b) s -> f g b s", b=4)[:, :, 2, :]
        ht1b = xb[0:32, :, :].rearrange("f (g b) s -> f g b s", b=4)[:, :, 3, :]

        # ---- tp2 feature-major; psum evicted by ACT, added by Pool ----
        htA_ps = tpsA.tile([P, MS], F32, tag="tpA")
        nc.tensor.matmul(htA_ps[:], W["WtttA"][:], mt_rhs, start=True, stop=True)
        htB_ps = tpsB.tile([32, MS], F32, tag="tpB")
        nc.tensor.matmul(htB_ps[:], W["WtttB"][:], mt_rhs, start=True, stop=True)
        htA = fm.tile([P, MS], F16, tag="htAf")
        nc.vector.tensor_tensor(htA[:].rearrange("f (g s) -> f g s", g=G), htA_ps[:].rearrange("f (g s) -> f g s", g=G), ht1a, OP.add)
        pk96 = fm.tile([96, MS], F16, tag="pk96")
        nc.vector.tensor_tensor(pk96[0:32, :].rearrange("f (g s) -> f g s", g=G), htB_ps[:].rearrange("f (g s) -> f g s", g=G), ht1b, OP.add)

        hs_ps = tpsB.tile([32, MS], F32, tag="tpB")
        nc.tensor.matmul(hs_ps[:], W["Wtts"][:], gt_rhs, start=True, stop=True)
        hs_fm = fm.tile([32, MS], F16, tag="hsf")
        nc.vector.tensor_tensor(hs_fm[:].rearrange("f (g s) -> f g s", g=G), hs_ps[:].rearrange("f (g s) -> f g s", g=G), h1t, OP.add)

        al_ps = tpsB.tile([32, MS], F32, tag="tpB")
        nc.tensor.matmul(al_ps[:], W["M2"][:], hs_fm[:], start=True, stop=True)
        nc.scalar.activation(pk96[64:96, :], al_ps[:], AF.Copy)

        g2A_ps = tpsA.tile([P, MS], F32, tag="tpA")
        nc.tensor.matmul(g2A_ps[:], W["W2A"][:], htA[:], start=True, stop=True)
        g2B_ps = tpsB.tile([32, MS], F32, tag="tpB")
        nc.tensor.matmul(g2B_ps[:], W["W2B"][:], pk96[0:32, :], start=True, stop=True)
        g2A = fm.tile([P, MS], F16, tag="g2Af")
        nc.scalar.activation(g2A[:], g2A_ps[:], AF.Copy)
        nc.scalar.activation(pk96[32:64, :], g2B_ps[:], AF.Copy)

        # ---- XBAR back ----
        xhA = fm.tile([P, G, P], F16, tag="xhA")
        nc.sync.dma_start_transpose(xhA[:], htA[:])
        xgA = fm.tile([P, G, P], F16, tag="xgA")
        nc.sync.dma_start_transpose(xgA[:], g2A[:])
        xp = fm.tile([P, G, 96], F16, tag="xp")
        nc.sync.dma_start_transpose(xp[:], pk96[:])

        htb4 = sb.tile([P, G, 160], F16, tag="htb4")
        nc.gpsimd.tensor_copy(htb4[:, :, 0:128], xhA[:])
        nc.gpsimd.tensor_copy(htb4[:, :, 128:160], xp[:, :, 0:32])
        g2b4 = sb.tile([P, G, 160], F16, tag="g2b4")
        nc.gpsimd.tensor_copy(g2b4[:, :, 0:128], xgA[:])
        nc.gpsimd.tensor_copy(g2b4[:, :, 128:160], xp[:, :, 32:64])

        qq4 = zq.tile([P, G, 800], F16, tag="qq4")
        for g in range(G):
            g2v = g2b4[:, g, :].rearrange("p (i v) -> p i v", i=5).unsqueeze(2).to_broadcast((P, 5, 5, 32))
            htv = htb4[:, g, :].rearrange("p (j v) -> p j v", j=5).unsqueeze(1).to_broadcast((P, 5, 5, 32))
            eng = nc.gpsimd if g < 2 else nc.vector
            eng.tensor_tensor(qq4[:, g, :].rearrange("p (i j v) -> p i j v", i=5, j=5), g2v, htv, OP.mult)
        Q4 = sb.tile([P, G, 25], F16, tag="Q4")
        qqh = zq.tile([P, G, 25, 16], F16, tag="qqh")
        qv = qq4[:].rearrange("p g (c v) -> p (g c) v", v=32)
        nc.gpsimd.tensor_tensor(
            qqh[:].rearrange("p g c v -> p (g c) v"), qv[:, :, 0:16], qv[:, :, 16:32], OP.add)
        nc.vector.tensor_reduce(
            Q4[:], qqh[:].rearrange("p g c v -> p (g c) v"), axis=AX.X, op=OP.add)

        q10 = zq.tile([P, G, 125], F16, tag="q10")
        for g in range(G):
            Qb = Q4[:, g, :].unsqueeze(1).to_broadcast((P, 5, 25))
            cv = W["CrepC"][:].rearrange("p (k c) -> p k c", k=5)
            eng = nc.gpsimd if g < 2 else nc.vector
            eng.tensor_tensor(q10[:, g, :].rearrange("p (k c) -> p k c", k=5), Qb, cv, OP.mult)
        q12 = zq.tile([P, G, 160], F16, tag="q12")
        for g in range(G):
            alb = xp[:, g, 64:96].unsqueeze(1).to_broadcast((P, 5, 32))
            htk = htb4[:, g, :].rearrange("p (k v) -> p k v", k=5)
            eng = nc.gpsimd if g < 2 else nc.vector
            eng.tensor_tensor(q12[:, g, :].rearrange("p (k v) -> p k v", k=5), alb, htk, OP.mult)
        o14 = sb.tile([P, G, 5], F32, tag="o14")
        nc.vector.tensor_reduce(
            o14[:], q10[:].rearrange("p g (c j) -> p (g c) j", j=25), axis=AX.X, op=OP.add)
        o24 = sb.tile([P, G, 5], F32, tag="o24")
        nc.vector.tensor_reduce(
            o24[:], q12[:].rearrange("p g (c v) -> p (g c) v", v=32), axis=AX.X, op=OP.add)
        out4 = io_pool.tile([P, G, 5], F32, tag="out4")
        nc.gpsimd.tensor_add(out4[:], o14[:], o24[:])
        nc.sync.dma_start(out_d[r0: r0 + MS, :].rearrange("(g p) c -> p g c", g=G), out4[:])


def build_program(n_macro=NMACRO):
    nc = bacc.Bacc("TRN2", target_bir_lowering=False, debug=False,
                   enable_asserts=False, num_devices=NCORES)
    rows = n_macro * MS
    io = {
        "s": nc.dram_tensor("s", [rows, NS], F16, kind="ExternalInput").ap(),
        "t": nc.dram_tensor("t", [rows, 25], F16, kind="ExternalInput").ap(),
        "stT": nc.dram_tensor("stT", [89, rows], F16, kind="ExternalInput").ap(),
        "out": nc.dram_tensor("out", [rows, 5], F32, kind="ExternalOutput").ap(),
        "w": {name: nc.dram_tensor(name, list(shp), F16, kind="ExternalInput").ap()
              for name, shp in WEIGHT_SHAPES.items()},
    }
    with tile.TileContext(nc) as tc:
        with ExitStack() as ctx:
            _tile_body(ctx, tc, io, n_macro)
    nc.compile()
    return nc


def make_in_maps(
    scalars, kernel_t2_sum, mc_t2, coulomb_t2, bs_t2, mopac_coulomb_t2,
    w1_sss, w1_stt, w1_tst, w1_tts, w1_ttt, w2_stt, w2_tst, w2_ttt,
):
    wmap = prep_weights(w1_sss, w1_stt, w1_tst, w1_tts, w1_ttt, w2_stt, w2_tst, w2_ttt)
    s = np.ascontiguousarray(np.asarray(scalars, np.float16))
    t = np.stack(
        [np.asarray(kernel_t2_sum, np.float32), np.asarray(mc_t2, np.float32),
         np.asarray(coulomb_t2, np.float32), np.asarray(bs_t2, np.float32),
         np.asarray(mopac_coulomb_t2, np.float32)], axis=1,
    ).reshape(B, 25).astype(np.float16)
    in_maps = []
    for c in range(NCORES):
        sh = s[c * BPC: (c + 1) * BPC]
        th = t[c * BPC: (c + 1) * BPC]
        stT = np.concatenate([sh.T, th.T], axis=0)
        m = {"s": sh, "t": np.ascontiguousarray(th), "stT": np.ascontiguousarray(stT)}
        m.update(wmap)
        in_maps.append(m)
    return in_maps


_CACHED_NC = None


def kernel(
    scalars, kernel_t2_sum, mc_t2, coulomb_t2, bs_t2, mopac_coulomb_t2,
    w1_sss, w1_stt, w1_tst, w1_tts, w1_ttt, w2_stt, w2_tst, w2_ttt,
):
    global _CACHED_NC
    if _CACHED_NC is None:
        _CACHED_NC = build_program(NMACRO)
    nc = _CACHED_NC
    in_maps = make_in_maps(
        scalars, kernel_t2_sum, mc_t2, coulomb_t2, bs_t2, mopac_coulomb_t2,
        w1_sss, w1_stt, w1_tst, w1_tts, w1_ttt, w2_stt, w2_tst, w2_ttt,
    )
    res = run_bass_kernel_spmd(nc, in_maps, list(range(NCORES)))
    out = np.concatenate([res.results[c]["out"] for c in range(NCORES)], axis=0)
    return out.astype(np.float32)
